# revision 12
# baseline (speedup 1.0000x reference)
"""GATPlanner Trainium2 kernel: 8 NeuronCores, SPMD.

Sharding: batch b -> core group (cores 0-3: b=0, cores 4-7: b=1); within a
group, the N x N attention is sharded by i-row blocks of 640 nodes per core
(node dim padded 2500 -> 2560). Each GAT layer computes local y columns,
then AllGather within the group rebuilds the full activation.

Math per GAT layer (head p): with s1 = a1.x, s2 = a2.x,
  num[j,i] = exp(lrelu(s1[i]+s2[j]) - Mg) * S^T[j,i]
  out[f,i] = sum_j [1 | X@W1]^T[j,f] num[j,i]   (PE; row 0 = softmax denom d)
  y[pf,i]  = (X@W0)^T[pf,i] + b[f] + out[1+f,i] / d[i]
The global per-head shift Mg = lrelu(max s1 + max s2) cancels exactly in the
softmax ratio; it only guards exp overflow.
"""
import numpy as np

import concourse.bacc as bacc
import concourse.tile as tile
from concourse import mybir, bass_utils

F32 = mybir.dt.float32
BF16 = mybir.dt.bfloat16
AF = mybir.ActivationFunctionType
ALU = mybir.AluOpType

# dense-path dtype (exp values / masked numerators / folded weights)
USE_BF16 = True
DT = BF16 if USE_BF16 else F32

NCORES = 8
GROUPS = [[0, 1, 2, 3], [4, 5, 6, 7]]
N = 2500
NP = 2560          # padded nodes
NT = NP // 128     # 20 j-tiles
NSH = NP // 4      # 640 i-cols per core
P = 4              # heads
IBS = [(0, 512), (512, 128)]   # i-blocks within the local shard

# gi: (name, G, F, s_idx)
GATS = {
    "down0": (32, 16, 0),
    "down1": (64, 8, 1),
    "up0": (32, 16, 1),
    "up1": (64, 32, 0),
    "sc0": (32, 32, 0),
    "sc1": (64, 16, 1),
}
# each pass: (s_idx, [gat names]); 2-gat passes share the S^T tile stream
PASSES = [(0, ["down0"]), (1, ["down1"]), (1, ["up0", "sc1"]), (0, ["up1", "sc0"])]
SRC = {"down0": "f0", "down1": "f1", "up0": "f2", "sc1": "f1", "up1": "g1", "sc0": "f0"}
DST = {0: "f1", 1: "f2", 2: "g1", 3: "g2"}
XDIM = {"f0": 32, "f1": 64, "f2": 32, "g1": 64, "g2": 128}

DEBUG = False


def _conv_taps(w):
    """OIHW (O,I,3,3) -> [I, 9, O] tap-major for lhsT slices."""
    o, i, kh, kw = w.shape
    return np.ascontiguousarray(w.transpose(1, 2, 3, 0).reshape(i, kh * kw, o))


def host_prep(inputs):
    """Build the 8 per-core input maps from the full model inputs."""
    import ml_dtypes
    np_dt = ml_dtypes.bfloat16 if USE_BF16 else np.float32

    shared = {}
    for gname, (G, F, _s) in GATS.items():
        a = np.asarray(inputs[gname + "_a"])      # (4, 2G)
        W = np.asarray(inputs[gname + "_W"])      # (4, 2, G, F)
        b = np.asarray(inputs[gname + "_b"])      # (F,)
        aT = np.zeros((G, 8), np.float32)
        aT[:, 0:4] = a[:, :G].T
        aT[:, 4:8] = a[:, G:].T
        w1s = W[:, 1].transpose(1, 0, 2).reshape(G, 4 * F)
        w0s = W[:, 0].transpose(1, 0, 2).reshape(G, 4 * F)
        b4 = np.tile(b, 4)[:, None]
        shared[f"aT_{gname}"] = np.ascontiguousarray(aT, np.float32)
        shared[f"w1s_{gname}"] = np.ascontiguousarray(w1s, np.float32)
        shared[f"w0s_{gname}"] = np.ascontiguousarray(w0s, np.float32)
        shared[f"b4_{gname}"] = np.ascontiguousarray(b4, np.float32)

    shared["ew1"] = _conv_taps(np.asarray(inputs["enc_w1"]))
    shared["eb1"] = np.asarray(inputs["enc_b1"])[:, None].astype(np.float32)
    shared["ew2"] = _conv_taps(np.asarray(inputs["enc_w2"]))
    shared["eb2"] = np.asarray(inputs["enc_b2"])[:, None].astype(np.float32)
    shared["dw1"] = _conv_taps(np.asarray(inputs["dec_w1"]))
    shared["db1"] = np.asarray(inputs["dec_b1"])[:, None].astype(np.float32)
    shared["dw2"] = _conv_taps(np.asarray(inputs["dec_w2"]))
    shared["db2"] = np.asarray(inputs["dec_b2"])[:, None].astype(np.float32)
    shared["caw1T"] = np.ascontiguousarray(np.asarray(inputs["ca_w1"]).T, np.float32)
    shared["cab1"] = np.asarray(inputs["ca_b1"])[:, None].astype(np.float32)
    shared["caw2T"] = np.ascontiguousarray(np.asarray(inputs["ca_w2"]).T, np.float32)
    shared["cab2"] = np.asarray(inputs["ca_b2"])[:, None].astype(np.float32)

    w1T = np.zeros((NP, 512), np.float32)
    w1T[:N] = np.asarray(inputs["mlp_w1"]).T
    shared["mw1t"] = np.ascontiguousarray(
        w1T.reshape(NT, 128, 512).transpose(1, 0, 2))
    shared["mb1"] = np.asarray(inputs["mlp_b1"])[None, :].astype(np.float32)
    shared["mw2t"] = np.ascontiguousarray(
        np.asarray(inputs["mlp_w2"]).T.reshape(4, 128, 5).transpose(1, 0, 2))
    shared["mb2"] = np.asarray(inputs["mlp_b2"])[:, None].astype(np.float32)

    x = np.asarray(inputs["x"])                       # (2, 3, 50, 50)
    Slist = np.asarray(inputs["Slist"])               # (2, 2, 2500, 2500)
    STb = np.zeros((2, 2, NP, NP), np_dt)
    for b in range(2):
        for s in range(2):
            STb[b, s, :N, :N] = Slist[b, s].T.astype(np_dt)

    in_maps = []
    for c in range(NCORES):
        b, r = c // 4, c % 4
        m = dict(shared)
        m["xb"] = np.ascontiguousarray(x[b].reshape(3, N), np.float32)
        m["ST"] = np.ascontiguousarray(STb[b, :, :, r * NSH:(r + 1) * NSH])
        sel = np.zeros((1, 4), np.float32)
        sel[0, r] = 1.0
        m["sel"] = sel
        in_maps.append(m)
    return in_maps


def _declare(nc):
    d = {}
    def di(name, shape, dt=F32):
        d[name] = nc.dram_tensor(name, list(shape), dt, kind="ExternalInput")
    di("xb", (3, N))
    di("ST", (2, NP, NSH), DT)
    di("sel", (1, 4))
    for gname, (G, F, _s) in GATS.items():
        di(f"aT_{gname}", (G, 8))
        di(f"w1s_{gname}", (G, 4 * F))
        di(f"w0s_{gname}", (G, 4 * F))
        di(f"b4_{gname}", (4 * F, 1))
    di("ew1", (3, 9, 32)); di("eb1", (32, 1))
    di("ew2", (32, 9, 32)); di("eb2", (32, 1))
    di("dw1", (128, 9, 32)); di("db1", (32, 1))
    di("dw2", (32, 9, 1)); di("db2", (1, 1))
    di("caw1T", (128, 128)); di("cab1", (128, 1))
    di("caw2T", (128, 128)); di("cab2", (128, 1))
    di("mw1t", (128, NT, 512)); di("mb1", (1, 512))
    di("mw2t", (128, 4, 5)); di("mb2", (5, 1))
    d["out5"] = nc.dram_tensor("out5", [1, 5], F32, kind="ExternalOutput")
    if DEBUG:
        for nm, g in XDIM.items():
            d["dbg_" + nm] = nc.dram_tensor("dbg_" + nm, [g, NP], F32,
                                            kind="ExternalOutput")
        d["dbg_att"] = nc.dram_tensor("dbg_att", [128, 2], F32, kind="ExternalOutput")
        d["dbg_flat"] = nc.dram_tensor("dbg_flat", [1, NP], F32, kind="ExternalOutput")
        d["dbg_m1"] = nc.dram_tensor("dbg_m1", [1, 512], F32, kind="ExternalOutput")
        d["dbg_dp2"] = nc.dram_tensor("dbg_dp2", [32, 52 * 52], F32, kind="ExternalOutput")
    return d


def _load_weights(nc, wt, d):
    """DMA all small weights into SBUF tiles; returns dict of tiles."""
    t = {}
    def ld(name, shape, dt=F32):
        tl = wt.tile(list(shape), dt, tag="w_" + name, name="w_" + name)
        nc.sync.dma_start(tl[...], d[name][...])
        t[name] = tl
    for gname, (G, F, _s) in GATS.items():
        ld(f"aT_{gname}", (G, 8))
        ld(f"w1s_{gname}", (G, 4 * F))
        ld(f"w0s_{gname}", (G, 4 * F))
        ld(f"b4_{gname}", (4 * F, 1))
    ld("ew1", (3, 9, 32)); ld("eb1", (32, 1))
    ld("ew2", (32, 9, 32)); ld("eb2", (32, 1))
    ld("dw1", (128, 9, 32)); ld("db1", (32, 1))
    ld("dw2", (32, 9, 1)); ld("db2", (1, 1))
    ld("caw1T", (128, 128)); ld("cab1", (128, 1))
    ld("caw2T", (128, 128)); ld("cab2", (128, 1))
    ld("mb1", (1, 512)); ld("mw2t", (128, 4, 5)); ld("mb2", (5, 1))
    return t


def _conv(nc, pp, taps_w, bias, src_pad, dst_write, cin, cout, relu=True):
    """3x3 conv over a [cin, 52, 52] padded tile; dst_write(rc, ap) consumes
    the [cout, 10, 50] activated chunk for row-chunk rc."""
    for rc in range(5):
        acc = pp.tile([cout, 500], F32, tag="convacc", bufs=2)
        for tap in range(9):
            dy, dx = tap // 3, tap % 3
            rhs = src_pad[:, rc * 10 + dy: rc * 10 + dy + 10, dx:dx + 50]
            nc.tensor.matmul(acc[...], taps_w[:, tap, :], rhs,
                             start=(tap == 0), stop=(tap == 8))
        dst_write(rc, acc)


def build_program():
    nc = bacc.Bacc("TRN2", target_bir_lowering=False, debug=False,
                   enable_asserts=False, num_devices=NCORES)
    d = _declare(nc)

    with tile.TileContext(nc) as tc:
        with (
            tc.tile_pool(name="wt", bufs=1) as wt,      # weights + persistent X
            tc.tile_pool(name="dram", bufs=1, space="DRAM") as dram,
        ):
            w = _load_weights(nc, wt, d)

            # persistent activations (feature-major, full width) + local shards
            X = {nm: wt.tile([g, NP], F32, tag="X" + nm, name="X" + nm)
                 for nm, g in XDIM.items()}
            XL = {nm: wt.tile([g, NSH], F32, tag="XL" + nm, name="XL" + nm)
                  for nm, g in XDIM.items() if nm != "g2"}

            al02 = wt.tile([128, 1], F32, tag="al02")
            nc.vector.memset(al02[...], 0.2)

            # rank one-hot -> broadcast [128, 4]
            sel1 = wt.tile([1, 4], F32, tag="sel1")
            nc.sync.dma_start(sel1[...], d["sel"][...])
            selb = wt.tile([128, 4, 1], F32, tag="selb")
            nc.gpsimd.partition_broadcast(selb[:, :, 0], sel1[0:1, :])

            # ---------------- encoder ----------------
            with (
                tc.tile_pool(name="enc", bufs=1) as enc,
                tc.tile_pool(name="encp", bufs=1, space="PSUM") as encp,
            ):
                xraw = enc.tile([3, N], F32, tag="xraw")
                nc.sync.dma_start(xraw[...], d["xb"][...])
                xp1 = enc.tile([3, 52, 52], F32, tag="xp1")
                nc.vector.memset(xp1[...], 0.0)
                nc.scalar.activation(xp1[:, 1:51, 1:51],
                                     xraw[:].rearrange("c (h w) -> c h w", h=50),
                                     AF.Sigmoid)
                xp2 = enc.tile([32, 52, 52], F32, tag="xp2")
                nc.vector.memset(xp2[...], 0.0)

                def w1_enc(rc, acc):
                    nc.scalar.activation(
                        xp2[:, 1 + rc * 10: 11 + rc * 10, 1:51],
                        acc[:].rearrange("o (h w) -> o h w", h=10),
                        AF.Relu, bias=w["eb1"][:, 0:1])
                _conv(nc, encp, w["ew1"], w["eb1"], xp1, w1_enc, 3, 32)

                nc.vector.memset(X["f0"][:, N:], 0.0)

                def w2_enc(rc, acc):
                    nc.scalar.activation(X["f0"][:, rc * 500:(rc + 1) * 500],
                                         acc[...], AF.Relu, bias=w["eb2"][:, 0:1])
                _conv(nc, encp, w["ew2"], w["eb2"], xp2, w2_enc, 32, 32)

                # f0 local shard via mask-reduce with the rank one-hot
                tmp = enc.tile([32, 4, NSH], F32, tag="encmask")
                nc.vector.tensor_tensor(
                    tmp[...], X["f0"][:].rearrange("g (r i) -> g r i", r=4),
                    selb[0:32, :, :].to_broadcast((32, 4, NSH)), ALU.mult)
                nc.vector.tensor_reduce(XL["f0"][...],
                                        tmp[:].rearrange("g r i -> g i r"),
                                        axis=mybir.AxisListType.X, op=ALU.add)

            # ---------------- GAT passes ----------------
            for pi, (sidx, gnames) in enumerate(PASSES):
                _gat_pass(nc, tc, d, w, X, XL, al02, pi, sidx, gnames, dram)
                if DEBUG:
                    nm = DST[pi]
                    nc.sync.dma_start(d["dbg_" + nm][...], X[nm][...])

            # ---------------- channel attention + decoder + MLP ----------------
            with (
                tc.tile_pool(name="hd", bufs=1) as hd,
                tc.tile_pool(name="hdp", bufs=1, space="PSUM") as hdp,
            ):
                g2 = X["g2"]
                att = hd.tile([128, 1], F32, tag="att")
                nc.vector.reduce_max(att[...], g2[:, 0:N], axis=mybir.AxisListType.X)
                pa = hdp.tile([128, 1], F32, tag="pa")
                nc.tensor.matmul(pa[...], w["caw1T"][...], att[...],
                                 start=True, stop=True)
                h1 = hd.tile([128, 1], F32, tag="h1")
                nc.scalar.activation(h1[...], pa[...], AF.Relu, bias=w["cab1"][:, 0:1])
                pb2 = hdp.tile([128, 1], F32, tag="pb2")
                nc.tensor.matmul(pb2[...], w["caw2T"][...], h1[...],
                                 start=True, stop=True)
                a2 = hd.tile([128, 1], F32, tag="a2")
                nc.scalar.activation(a2[...], pb2[...], AF.Sigmoid,
                                     bias=w["cab2"][:, 0:1])

                dp1 = hd.tile([128, 52, 52], F32, tag="dp1")
                nc.vector.memset(dp1[...], -999.0)
                nc.vector.tensor_scalar(dp1[:, 1:51, 1:51],
                                        g2[:, 0:N].rearrange("g (h w) -> g h w", h=50),
                                        a2[:, 0:1], None, ALU.mult)
                dp2 = hd.tile([32, 52, 52], F32, tag="dp2")
                nc.vector.memset(dp2[...], -999.0)

                def w1_dec(rc, acc):
                    nc.scalar.activation(dp2[:, 1 + rc * 10: 11 + rc * 10, 1:51],
                                         acc[:].rearrange("o (h w) -> o h w", h=10),
                                         AF.Relu, bias=w["db1"][:, 0:1])
                _conv(nc, hdp, w["dw1"], w["db1"], dp1, w1_dec, 128, 32)

                flat = hd.tile([1, NP], F32, tag="flat")
                nc.vector.memset(flat[...], 0.0)

                def w2_dec(rc, acc):
                    nc.scalar.activation(flat[0:1, rc * 500:(rc + 1) * 500],
                                         acc[...], AF.Relu, bias=w["db2"][:, 0:1])
                _conv(nc, hdp, w["dw2"], w["db2"], dp2, w2_dec, 32, 1)

                # MLP: flat [1, 2500] @ w1.T -> [1, 512] (k-chunks on partitions)
                flatc = hd.tile([128, NT], F32, tag="flatc")
                for t in range(NT):
                    nc.sync.dma_start(flatc[:, t:t + 1],
                                      flat[0:1, t * 128:(t + 1) * 128])
                mw1 = hd.tile([128, NT, 512], F32, tag="mw1")
                nc.sync.dma_start(mw1[...], d["mw1t"][...])
                m1p = hdp.tile([1, 512], F32, tag="m1p")
                for t in range(NT):
                    nc.tensor.matmul(m1p[...], flatc[:, t:t + 1], mw1[:, t, :],
                                     start=(t == 0), stop=(t == NT - 1))
                m1 = hd.tile([1, 512], F32, tag="m1")
                nc.vector.tensor_tensor(m1[...], m1p[...], w["mb1"][...], ALU.add)
                nc.scalar.activation(m1[...], m1[...], AF.Relu)
                mstage = hd.tile([128, 4], F32, tag="mstage")
                for ko in range(4):
                    nc.sync.dma_start(mstage[:, ko:ko + 1],
                                      m1[0:1, ko * 128:(ko + 1) * 128])
                o5p = hdp.tile([5, 1], F32, tag="o5p")
                for ko in range(4):
                    nc.tensor.matmul(o5p[...], w["mw2t"][:, ko, :],
                                     mstage[:, ko:ko + 1],
                                     start=(ko == 0), stop=(ko == 3))
                o5 = hd.tile([5, 1], F32, tag="o5")
                nc.scalar.activation(o5[...], o5p[...], AF.Sigmoid,
                                     bias=w["mb2"][:, 0:1])
                nc.sync.dma_start(d["out5"][0:1, :], o5[:, 0:1])
                if DEBUG:
                    nc.sync.dma_start(d["dbg_att"][:, 0:1], att[...])
                    nc.sync.dma_start(d["dbg_att"][:, 1:2], a2[...])
                    nc.sync.dma_start(d["dbg_flat"][...], flat[...])
                    nc.sync.dma_start(d["dbg_m1"][...], m1[...])
                    nc.sync.dma_start(d["dbg_dp2"][...],
                                      dp2[:].rearrange("c h w -> c (h w)"))

    nc.compile()
    return nc


def _gat_pass(nc, tc, d, w, X, XL, al02, pi, sidx, gnames, dram):
    specs = [(g, *GATS[g]) for g in gnames]   # (name, G, F, s)
    ytiles = {}

    with tc.tile_pool(name=f"gp{pi}", bufs=1) as gp:
      with tc.tile_pool(name=f"gpA{pi}", bufs=1, space="PSUM") as ppA:
        # ---- per-gat prep: X1w, s_nm, s_fm, Mg, s_loc, xw0t ----
        prep = {}
        for gname, G, F, _s in specs:
            Xs, Xl = X[SRC[gname]], XL[SRC[gname]]
            Fp1 = F + 1
            x1w = gp.tile([128, NT, 4, Fp1], DT, tag=f"x1w{gname}", name=f"x1w{gname}")
            nc.vector.memset(x1w[:, :, :, F:F + 1], 1.0)
            s_nm = gp.tile([128, NT, 8], F32, tag=f"snm{gname}")
            for t in range(NT):
                lhs = Xs[:, t * 128:(t + 1) * 128]
                p1 = ppA.tile([128, 4 * F], F32, tag="ppbuild", bufs=2)
                nc.tensor.matmul(p1[...], lhs, w[f"w1s_{gname}"][...],
                                 start=True, stop=True)
                nc.vector.tensor_copy(x1w[:, t, :, 0:F],
                                      p1[:].rearrange("p (h f) -> p h f", h=4))
                p2 = ppA.tile([128, 8], F32, tag="ppsnm", bufs=2)
                nc.tensor.matmul(p2[...], lhs, w[f"aT_{gname}"][...],
                                 start=True, stop=True)
                nc.vector.tensor_copy(s_nm[:, t, :], p2[...])

            mparts = gp.tile([8, 5], F32, tag=f"mparts{gname}")
            for c in range(5):
                p3 = ppA.tile([8, 512], F32, tag="ppsfm", bufs=2)
                nc.tensor.matmul(p3[...], w[f"aT_{gname}"][...],
                                 Xs[:, c * 512:(c + 1) * 512], start=True, stop=True)
                nc.vector.reduce_max(mparts[:, c:c + 1], p3[...],
                                     axis=mybir.AxisListType.X)

            m8 = gp.tile([8, 1], F32, tag=f"m8{gname}")
            nc.vector.reduce_max(m8[...], mparts[...], axis=mybir.AxisListType.X)
            msrow = gp.tile([1, 8], F32, tag=f"msrow{gname}")
            nc.sync.dma_start(msrow[0:1, :], m8[:, 0:1])
            mg1 = gp.tile([1, 4], F32, tag=f"mg1{gname}")
            nc.vector.tensor_tensor(mg1[...], msrow[0:1, 0:4], msrow[0:1, 4:8],
                                    ALU.add)
            nc.scalar.activation(mg1[...], mg1[...], AF.Prelu, alpha=al02[0:1, 0:1])
            nc.vector.tensor_scalar_mul(mg1[...], mg1[...], -1.0)
            mgb = gp.tile([128, 4], F32, tag=f"mgb{gname}")
            for p in range(4):
                nc.gpsimd.partition_broadcast(mgb[:, p:p + 1], mg1[0:1, p:p + 1])

            sl4 = gp.tile([4, NSH], F32, tag=f"sl4{gname}")
            for (c0, cw) in IBS:
                p4 = ppA.tile([8, 512], F32, tag="ppsfm", bufs=2)
                nc.tensor.matmul(p4[0:4, :cw], w[f"aT_{gname}"][:, 0:4],
                                 Xl[:, c0:c0 + cw], start=True, stop=True)
                nc.vector.tensor_copy(sl4[:, c0:c0 + cw], p4[0:4, :cw])
            sl1 = gp.tile([1, 4, NSH], F32, tag=f"sl1{gname}")
            nc.sync.dma_start(sl1[0:1, :, :], sl4[...])

            xw0t = gp.tile([32, 4, NSH], F32, tag=f"xw0{gname}")
            for p in range(4):
                for (c0, cw) in IBS:
                    p5 = ppA.tile([32, 512], F32, tag="ppxw0", bufs=2)
                    nc.tensor.matmul(p5[:F, :cw],
                                     w[f"w0s_{gname}"][:, p * F:(p + 1) * F],
                                     Xl[:, c0:c0 + cw], start=True, stop=True)
                    nc.scalar.activation(xw0t[0:F, p, c0:c0 + cw], p5[:F, :cw],
                                         AF.Identity,
                                         bias=w[f"b4_{gname}"][0:F, 0:1])

            ytiles[gname] = gp.tile([4 * F, NSH], F32, tag=f"y{gname}", name=f"y{gname}")
            prep[gname] = (x1w, s_nm, mgb, sl1, xw0t, G, F)

      # ---- main i-loop ----  (prep PSUM pool closed above)
      if True:
        with tc.tile_pool(name=f"gpB{pi}", bufs=1, space="PSUM") as ppB:
            for (ib0, ibw) in IBS:
                s1b = {}
                zps = {}
                for gname, G, F, _s in specs:
                    (x1w, s_nm, mgb, sl1, xw0t, G, F) = prep[gname]
                    for p in range(4):
                        sb_t = gp.tile([128, 512], F32, name=f"s1b{gname}{p}",
                                       tag=f"s1b{gname}{p}", bufs=1)
                        nc.gpsimd.partition_broadcast(
                            sb_t[:, :ibw], sl1[0:1, p, ib0:ib0 + ibw])
                        s1b[(gname, p)] = sb_t
                        zps[(gname, p)] = ppB.tile([F + 1, 512], F32, name=f"z{gname}{p}",
                                                   tag=f"z{gname}{p}", bufs=1)
                with tc.tile_pool(name=f"il{pi}{ib0}", bufs=1) as il:
                    for t in range(NT):
                        st_t = il.tile([128, 512], DT, tag="st", bufs=3)
                        nc.sync.dma_start(
                            st_t[:, :ibw],
                            d["ST"][sidx, t * 128:(t + 1) * 128, ib0:ib0 + ibw])
                        for gname, G, F, _s in specs:
                            (x1w, s_nm, mgb, sl1, xw0t, G, F) = prep[gname]
                            for p in range(4):
                                l_t = il.tile([128, 512], F32, tag="l", bufs=3)
                                nc.scalar.activation(
                                    l_t[:, :ibw], s1b[(gname, p)][:, :ibw],
                                    AF.Prelu, bias=s_nm[:, t, 4 + p: 5 + p],
                                    alpha=al02[:, 0:1])
                                e_t = il.tile([128, 512], DT, tag="e", bufs=3)
                                nc.scalar.activation(
                                    e_t[:, :ibw], l_t[:, :ibw], AF.Exp,
                                    bias=mgb[:, p:p + 1])
                                nm_t = il.tile([128, 512], DT, tag="nm", bufs=4)
                                nc.vector.tensor_tensor(
                                    nm_t[:, :ibw], e_t[:, :ibw], st_t[:, :ibw],
                                    ALU.mult)
                                nc.tensor.matmul(
                                    zps[(gname, p)][:, :ibw], x1w[:, t, p, :],
                                    nm_t[:, :ibw],
                                    start=(t == 0), stop=(t == NT - 1))
                    # ---- epilogue for this i-block ----
                    for gname, G, F, _s in specs:
                        (x1w, s_nm, mgb, sl1, xw0t, G, F) = prep[gname]
                        for p in range(4):
                            zb = il.tile([33, 512], F32, tag="zb", bufs=2)
                            nc.vector.tensor_copy(zb[:F + 1, :ibw],
                                                  zps[(gname, p)][:, :ibw])
                            dst_ = il.tile([1, 512], F32, tag="dst_", bufs=2)
                            nc.sync.dma_start(dst_[0:1, :ibw], zb[F:F + 1, :ibw])
                            dbc = il.tile([32, 512], F32, tag="dbc", bufs=2)
                            nc.gpsimd.partition_broadcast(dbc[:F, :ibw],
                                                          dst_[0:1, :ibw])
                            nc.vector.tensor_scalar_add(dbc[:F, :ibw],
                                                        dbc[:F, :ibw], 1e-30)
                            rec = il.tile([32, 512], F32, tag="rec", bufs=2)
                            nc.vector.reciprocal_approx_fast(rec[:F, :ibw],
                                                             dbc[:F, :ibw])
                            yh = il.tile([32, 512], F32, tag="yh", bufs=2)
                            nc.vector.tensor_tensor(yh[:F, :ibw],
                                                    zb[0:F, :ibw],
                                                    rec[:F, :ibw], ALU.mult)
                            nc.vector.tensor_tensor(
                                yh[:F, :ibw], yh[:F, :ibw],
                                xw0t[0:F, p, ib0:ib0 + ibw], ALU.add)
                            nc.sync.dma_start(
                                ytiles[gname][p * F:(p + 1) * F, ib0:ib0 + ibw],
                                yh[:F, :ibw])

        # ---- combine + allgather ----
        dst = DST[pi]
        gdim = XDIM[dst]
        if len(specs) == 2:
            ysum = ytiles[specs[0][0]]
            nc.vector.tensor_tensor(ysum[...], ysum[...],
                                    ytiles[specs[1][0]][...], ALU.add)
        else:
            ysum = ytiles[specs[0][0]]

        if dst != "g2":
            nc.vector.tensor_copy(XL[dst][...], ysum[...])
        gin = dram.tile([gdim, NSH], F32, tag=f"gin{pi}", name=f"gin{pi}")
        gout = dram.tile([4, gdim, NSH], F32, tag=f"gout{pi}", name=f"gout{pi}")
        nc.sync.dma_start(gin[...], ysum[...])
        nc.gpsimd.collective_compute(
            "AllGather", ALU.bypass, replica_groups=GROUPS,
            ins=[gin.opt()], outs=[gout.opt()])
        nc.sync.dma_start(X[dst][...], gout[:].rearrange("r g c -> g r c"))


_prog_cache = {}


def get_program():
    key = (USE_BF16, DEBUG)
    if key not in _prog_cache:
        _prog_cache[key] = build_program()
    return _prog_cache[key]


def kernel(**inputs) -> np.ndarray:
    in_maps = host_prep(inputs)
    nc = get_program()
    res = bass_utils.run_bass_kernel_spmd(nc, in_maps,
                                          core_ids=list(range(NCORES)))
    outs = res.results
    return np.stack([outs[0]["out5"][0], outs[4]["out5"][0]]).astype(np.float32)
